# revision 31
# baseline (speedup 1.0000x reference)
"""MinRNN Trainium2 kernel — quasi-DEER fixed-point iteration, v5 (fp16).

Model (per batch row):
    z_t = tanh(x_t @ W_in^T + b_in)
    u_t = sigmoid(s_{t-1} @ W_rec^T + z_t @ U_z^T + b_u)
    s_t = u_t * s_{t-1} + (1 - u_t) * z_t

Reformulated on the deviation m = s - z (with the convention z_{-1} = 0,
so m_{-1} = 0):

    pre_t = W_rec m_{t-1} + ct_t,   ct_t = U_z z_t + W_rec z_{t-1} + b_u
    u_t   = sigmoid(pre_t)
    m_t   = (dz_t + m_{t-1}) * u_t,  dz_t = z_{t-1} - z_t
    s_t   = z_t + m_t

Solved by fixed-point sweeps (quasi-DEER): freeze u from the previous
iterate, then the m-recurrence is solved EXACTLY by the HW linear-scan
instruction (tensor_tensor_scan computes state=(d0+state)*d1 in fp32).
Each sweep's GEMM uses 512-wide moving operands (vs 2-wide in a naive
sequential scan), amortizing W_rec weight loads.  m^0 = 0 makes sweep 1
GEMM-free: u^1 = sigmoid(ct) straight from the ct psum.  Scans run
512 elements wide (two GEMM tiles per scan) to amortize the ~0.5us
fixed cost per DVE instruction.

Everything is fp16 (same PE rate as bf16, 8x finer mantissa): K=4
effective sweeps converge to max-abs err ~6.7e-3 on HW (HW matches the
numpy bit-model for fp16), 3x inside the 2e-2 tolerance.  The final
s = z + m add runs on the host: the device DMAs the contiguous m and
zneg buffers out, so the last sweep has no device-side epilogue.
Data-parallel over batch: 8 cores x 2 rows.
"""

import numpy as np
import ml_dtypes

import concourse.bass as bass
import concourse.mybir as mybir
import concourse.tile as tile
import concourse.bacc as bacc
from concourse import bass_utils

AF = mybir.ActivationFunctionType
OP = mybir.AluOpType

B, T, I, H = 16, 2048, 512, 512
N_CORES = 8
BL = B // N_CORES          # batch rows per core (2)
KC = I // 128              # input-dim chunks (4)
HC = H // 128              # hidden-dim chunks (4)
TB = 256                   # t-steps per GEMM tile (512 moving columns)
K_SWEEPS = 4               # effective sweeps (first one is GEMM-free)

f32 = mybir.dt.float32
f16 = mybir.dt.float16


def build(t_steps: int = T, tb: int = TB, sweeps: int = K_SWEEPS,
          compile: bool = True):
    tb = min(tb, t_steps)
    assert t_steps % tb == 0
    T1 = t_steps + 1

    nc = bacc.Bacc("TRN2", target_bir_lowering=False, debug=False)

    # x pre-tiled on the host: [tile, 128, (k t b)] so each tile DMA is a
    # fully contiguous [128, KC*tb*BL] block (4KB runs per partition)
    xT = nc.dram_tensor("xT", [t_steps // tb, 128, KC * tb * BL], f16,
                        kind="ExternalInput")
    winT = nc.dram_tensor("winT", [KC, 128, H], f16, kind="ExternalInput")
    wrecT = nc.dram_tensor("wrecT", [HC, 128, H], f16, kind="ExternalInput")
    uzT = nc.dram_tensor("uzT", [HC, 128, H], f16, kind="ExternalInput")
    binNeg = nc.dram_tensor("binNeg", [HC, 128], f32, kind="ExternalInput")
    bu2 = nc.dram_tensor("bu2", [HC, 128], f32, kind="ExternalInput")
    ident = nc.dram_tensor("ident", [128, 128], f16, kind="ExternalInput")
    mOut = nc.dram_tensor("mOut", [128, KC, T1, BL], f16, kind="ExternalOutput")
    zOut = nc.dram_tensor("zOut", [128, HC, T1, BL], f16, kind="ExternalOutput")

    with tile.TileContext(nc) as tc:
        _body(tc, nc, xT, winT, wrecT, uzT, binNeg, bu2, ident, mOut, zOut,
              t_steps, tb, sweeps)

    if compile:
        nc.compile()
    return nc


def _body(tc, nc, xT, winT, wrecT, uzT, binNeg, bu2, ident, mOut, zOut,
          t_steps, tb, sweeps):
    from contextlib import ExitStack

    nt = t_steps // tb          # number of GEMM tiles
    tw = tb * BL                # moving columns per tile (<=512)
    T1 = t_steps + 1            # state slots (slot j = value at step j-1)
    sp = 2 if nt % 2 == 0 else 1   # GEMM tiles per scan group
    ng = nt // sp               # scan groups per sweep
    sb = sp * tb                # t-steps per scan group
    sw_cols = sp * tw           # columns per scan group

    with ExitStack() as ctx:
        cpool = ctx.enter_context(tc.tile_pool(name="consts", bufs=1))
        xpool = ctx.enter_context(tc.tile_pool(name="xin", bufs=2))
        pspool = ctx.enter_context(tc.tile_pool(name="ps", bufs=8, space="PSUM"))
        upool = ctx.enter_context(tc.tile_pool(name="u", bufs=2))

        # ---- constants ----
        w_in = cpool.tile([128, KC * H], f16, tag="w_in")
        w_rec = cpool.tile([128, HC * H], f16, tag="w_rec")
        u_z = cpool.tile([128, HC * H], f16, tag="u_z")
        # x tile 0 first — the first z-GEMM needs it plus w_in only
        xs0 = xpool.tile([128, KC * tb * BL], f16, tag="xs", name="xs0")
        nc.sync.dma_start(xs0[:], xT[0])
        for k in range(KC):
            nc.sync.dma_start(w_in[:, k * H:(k + 1) * H], winT[k])
        binS = cpool.tile([128, HC], f32, tag="binS")
        nc.sync.dma_start(binS[:], binNeg.ap().rearrange("c p -> p c"))
        buS = cpool.tile([128, HC], f32, tag="buS")
        nc.sync.dma_start(buS[:], bu2.ap().rearrange("c p -> p c"))
        for k in range(KC):
            nc.sync.dma_start(w_rec[:, k * H:(k + 1) * H], wrecT[k])
            nc.sync.dma_start(u_z[:, k * H:(k + 1) * H], uzT[k])
        idn = cpool.tile([128, 128], f16, tag="idn")
        nc.sync.dma_start(idn[:], ident[:])
        zzero = cpool.tile([128, 1], f32, tag="zzero")
        nc.vector.memset(zzero[:], 0.0)

        # ---- persistent whole-T tensors (T1 slot layout, slot 0 == 0) ----
        zneg = cpool.tile([128, HC * T1 * BL], f16, tag="zneg")   # -z
        ctil = cpool.tile([128, HC * t_steps * BL], f16, tag="ctil")
        dzb = cpool.tile([128, HC * t_steps * BL], f16, tag="dzb")
        mA = cpool.tile([128, KC * T1 * BL], f16, tag="mA")
        mB = cpool.tile([128, KC * T1 * BL], f16, tag="mB")
        m_bufs = [mA, mB]

        zn4 = zneg[:].rearrange("p (c t b) -> p c t b", c=HC, b=BL)
        dz4 = dzb[:].rearrange("p (c t b) -> p c t b", c=HC, b=BL)
        ct2 = ctil[:].rearrange("p (c f) -> p c f", c=HC)
        mv4 = [m[:].rearrange("p (k t b) -> p k t b", k=KC, b=BL)
               for m in m_bufs]

        nc.vector.memset(zn4[:, :, 0, :], 0.0)      # z_{-1} = 0
        for mv in mv4:
            nc.vector.memset(mv[:, :, 0, :], 0.0)   # m_{-1} = 0

        # contiguous 2D slice of the T1 layout: slots [j0, j0+ncols/BL)
        def zslot(c, j0, ncols):
            st = c * T1 * BL + j0 * BL
            return zneg[:, st:st + ncols]

        def scans(gi, gs, gw, wv, ut, dma_fn=None):
            """Scans over one group of gw t-steps: m_t = (dz_t+m_{t-1})*u_t."""
            u4 = ut[:].rearrange("p (c t b) -> p c t b", c=HC, b=BL)
            for cm in range(HC):
                for b in range(BL):
                    init = (zzero[:, 0:1] if gi == 0
                            else wv[:, cm, gs:gs + 1, b])
                    nc.vector.tensor_tensor_scan(
                        wv[:, cm, 1 + gs:1 + gs + gw, b],
                        dz4[:, cm, gs:gs + gw, b],
                        u4[:, cm, 0:gw, b],
                        init, op0=OP.add, op1=OP.mult)
            if dma_fn is not None:
                dma_fn(gs, gw)

        # ====== phase 1 (fused): per tile z, dz, ct; per group sweep-1 ===
        # zneg = -tanh(W_in x + b_in)
        # psum = U_z zneg_t + W_rec zneg_{t-1} = -(U_z z_t + W_rec z_{t-1})
        # ct   = -psum + b_u;  u^1 = sigmoid(ct)   (m^0 = 0)
        # sweep-1 scans per group: m^1 = linscan(dz, u^1) -> m_bufs[1]
        for gi in range(ng):
            gs = gi * sb
            ut = upool.tile([128, HC * sw_cols], f16, tag="u")
            u3 = ut[:].rearrange("p (c f) -> p c f", c=HC)
            for half in range(sp):
                ti = gi * sp + half
                ts = gs + half * tb
                if ti == 0:
                    xs = xs0
                else:
                    xs = xpool.tile([128, KC * tw], f16, tag="xs")
                    nc.sync.dma_start(xs[:], xT[ti])
                for cm in range(HC):
                    ps = pspool.tile([128, tw], f32, tag="ps", name=f"za{cm}")
                    for k in range(KC):
                        nc.tensor.matmul(
                            ps[:],
                            w_in[:, k * H + cm * 128:k * H + cm * 128 + 128],
                            xs[:, k * tw:(k + 1) * tw],
                            start=(k == 0), stop=(k == KC - 1))
                    nc.scalar.activation(zslot(cm, 1 + ts, tw), ps[:],
                                         AF.Tanh, bias=binS[:, cm:cm + 1],
                                         scale=-1.0)
                # dz_t = z_{t-1} - z_t = zneg_t - zneg_{t-1}  (once)
                nc.vector.tensor_sub(dz4[:, :, ts:ts + tb, :],
                                     zn4[:, :, 1 + ts:1 + ts + tb, :],
                                     zn4[:, :, ts:ts + tb, :])
                # stream z out for the host-side final s = z + m
                nc.sync.dma_start(zOut.ap()[:, :, 1 + ts:1 + ts + tb, :],
                                  zn4[:, :, 1 + ts:1 + ts + tb, :])
                for cm in range(HC):
                    ps = pspool.tile([128, tw], f32, tag="ps", name=f"cb{cm}")
                    for k in range(HC):
                        nc.tensor.matmul(
                            ps[:],
                            u_z[:, k * H + cm * 128:k * H + cm * 128 + 128],
                            zslot(k, 1 + ts, tw),
                            start=(k == 0), stop=False, skip_group_check=True)
                    for k in range(HC):
                        nc.tensor.matmul(
                            ps[:],
                            w_rec[:, k * H + cm * 128:k * H + cm * 128 + 128],
                            zslot(k, ts, tw),
                            start=False, stop=(k == HC - 1),
                            skip_group_check=True)
                    nc.scalar.activation(
                        ct2[:, cm, ts * BL:ts * BL + tw], ps[:],
                        AF.Identity, bias=buS[:, cm:cm + 1], scale=-1.0)
                    nc.scalar.activation(
                        u3[:, cm, half * tw:half * tw + tw], ps[:],
                        AF.Sigmoid, bias=buS[:, cm:cm + 1], scale=-1.0)
            scans(gi, gs, sb, mv4[1], ut)

        # ================= phase 2: GEMM sweeps 2..K =====================
        for sw in range(1, sweeps):
            rv = mv4[sw % 2]
            wv = mv4[(sw + 1) % 2]
            last = sw == sweeps - 1
            if last and sp == 2 and nt >= 4:
                # narrow trailing groups so the post-GEMM scan tail is short
                glist = [2] * ((nt - 2) // 2) + [1, 1]
            else:
                glist = [sp] * ng
            ti0 = 0
            for gi, gsz in enumerate(glist):
                gs = ti0 * tb
                ut = upool.tile([128, HC * sw_cols], f16, tag="u")
                u3 = ut[:].rearrange("p (c f) -> p c f", c=HC)
                for half in range(gsz):
                    ts = gs + half * tb
                    pss = [pspool.tile([128, tw], f32, tag="ps",
                                       name=f"pp{cm}")
                           for cm in range(HC)]
                    for cm in range(HC):
                        nc.tensor.matmul(pss[cm][:], idn[:],
                                         ct2[:, cm, ts * BL:ts * BL + tw],
                                         start=True, stop=False,
                                         skip_group_check=True)
                    for k in range(KC):
                        for cm in range(HC):
                            nc.tensor.matmul(
                                pss[cm][:],
                                w_rec[:, k * H + cm * 128:
                                      k * H + cm * 128 + 128],
                                rv[:, k, ts:ts + tb, :],
                                start=False, stop=(k == KC - 1),
                                skip_group_check=True)
                    for cm in range(HC):
                        nc.scalar.activation(u3[:, cm, half * tw:
                                                half * tw + tw],
                                             pss[cm][:], AF.Sigmoid)
                if last:
                    def mdma(qs, ww):
                        nc.sync.dma_start(mOut.ap()[:, :, 1 + qs:1 + qs + ww, :],
                                          wv[:, :, 1 + qs:1 + qs + ww, :])
                    scans(gi, gs, gsz * tb, wv, ut, dma_fn=mdma)
                else:
                    scans(gi, gs, gsz * tb, wv, ut)
                ti0 += gsz


_CACHED = {}


def _get_nc(t_steps=T, tb=TB):
    key = (t_steps, tb)
    if key not in _CACHED:
        _CACHED[key] = build(t_steps, tb)
    return _CACHED[key]


def make_in_maps(inputs, W_in, b_in, W_rec, U_z, b_u, t_steps=T):
    x = np.asarray(inputs, dtype=np.float32)
    winT_np = np.ascontiguousarray(
        np.asarray(W_in, np.float32).T.reshape(KC, 128, H)).astype(np.float16)
    wrecT_np = np.ascontiguousarray(
        np.asarray(W_rec, np.float32).T.reshape(HC, 128, H)).astype(np.float16)
    uzT_np = np.ascontiguousarray(
        np.asarray(U_z, np.float32).T.reshape(HC, 128, H)).astype(np.float16)
    binNeg_np = np.ascontiguousarray(
        (-np.asarray(b_in, np.float32)).reshape(HC, 128))
    bu_np = np.ascontiguousarray(np.asarray(b_u, np.float32).reshape(HC, 128))
    id_np = np.eye(128, dtype=np.float16)

    tb = min(TB, t_steps)
    nt = t_steps // tb
    in_maps = []
    for c in range(N_CORES):
        xc = x[c * BL:(c + 1) * BL, :t_steps, :]          # (BL, t, I)
        # -> [tile, 128, (k, t_local, b)] matching the SBUF tile layout
        xTc = np.ascontiguousarray(
            xc.reshape(BL, nt, tb, KC, 128).transpose(1, 4, 3, 2, 0)
        ).reshape(nt, 128, KC * tb * BL).astype(np.float16)
        in_maps.append({
            "xT": xTc, "winT": winT_np, "wrecT": wrecT_np, "uzT": uzT_np,
            "binNeg": binNeg_np, "bu2": bu_np, "ident": id_np,
        })
    return in_maps


def assemble_core(core_res, t_steps=T):
    """Host-side s = z + m from the device's m and zneg buffers."""
    m = np.asarray(core_res["mOut"])[:, :, 1:1 + t_steps, :]   # [128,KC,T,BL]
    zn = np.asarray(core_res["zOut"])[:, :, 1:1 + t_steps, :]  # [128,HC,T,BL]
    s = m.astype(np.float32) - zn.astype(np.float32)
    # [128, C, T, BL] -> [BL, T, C, 128] -> [BL, T, H]
    bl = s.shape[3]
    return np.ascontiguousarray(s.transpose(3, 2, 1, 0)).reshape(
        bl, t_steps, HC * 128)


def kernel(inputs, W_in, b_in, W_rec, U_z, b_u):
    nc = _get_nc()
    in_maps = make_in_maps(inputs, W_in, b_in, W_rec, U_z, b_u)
    res = bass_utils.run_bass_kernel_spmd(nc, in_maps, core_ids=list(range(N_CORES)))
    outs = [assemble_core(res.results[c]) for c in range(N_CORES)]
    return np.ascontiguousarray(np.concatenate(outs, axis=0), dtype=np.float32)
